# revision 1
# baseline (speedup 1.0000x reference)
"""AxialAttention Bass/Trainium2 kernel.

Problem: x [8, 128, 128, 128] (B, H, W, D), two axial multi-head self-attention
passes (8 heads, head dim 16): pass0 attends along H, pass1 attends along W;
output = pass0 + pass1.

Sharding: data-parallel over batch B across the 8 NeuronCores (core c gets
batch b=c). Each core computes both passes for its batch entirely on-chip.

Per-core dataflow (see inline comments):
  Phase 0: load x_b, cast fp16, DMA-xbar-transpose into xT [D=128, H*W] (SBUF).
  Per sequence s (128 seqs per pass, t=128, d=128):
    - qT/kT projections into 32-aligned even/odd head layouts (host-prepared
      zero-padded weight matrices), v natural projection.
    - dots^T per head via row-tiled (tile_position) K=16 matmuls.
    - one batched ACT exp (scale=1/4 folds the e^-0.5) -> expT fp16 + PSUM
      evacuation in the same op.
    - PV: lhsT=expT_h, rhs=[v_h | ones-col] -> out_nat [i, 17/head]; the ones
      column produces the softmax denominators per-partition for free.
    - reciprocal on the 8 denominator columns, broadcast-multiply (stride-0 AP)
      to normalize, -> ot fp16 [i, d'].
    - DMA-xbar-transpose ot -> otT [d', i]; final = otT.T @ Wo + bo via a K=1
      ones-row matmul for the bias.
    - pass0: DMA final PSUM -> DRAM staging; pass1: add staged + DMA out.
"""

import numpy as np
from contextlib import ExitStack

import concourse.bass as bass
import concourse.bacc as bacc
import concourse.tile as tile
from concourse import mybir
from concourse.bass_utils import run_bass_kernel_spmd

F16 = mybir.dt.float16
F32 = mybir.dt.float32

D = 128          # embedding dim
T = 128          # axial sequence length (H or W)
HEADS = 8
E = 16           # head dim
N_CORES = 8


def _axial_body(ctx: ExitStack, tc: "tile.TileContext", nseq: int):
    nc = tc.nc

    x = nc.dram_tensor("x", [T, T, D], F32, kind="ExternalInput")
    wq = nc.dram_tensor("wq", [2, 2, D, D], F16, kind="ExternalInput")
    wk = nc.dram_tensor("wk", [2, 2, D, D], F16, kind="ExternalInput")
    wv = nc.dram_tensor("wv", [2, D, D], F16, kind="ExternalInput")
    wo = nc.dram_tensor("wo", [2, D, D], F16, kind="ExternalInput")
    bo = nc.dram_tensor("bo", [2, 1, D], F16, kind="ExternalInput")
    ident = nc.dram_tensor("ident", [D, D], F16, kind="ExternalInput")
    out = nc.dram_tensor("out", [T, T, D], F32, kind="ExternalOutput")

    persist = ctx.enter_context(tc.tile_pool(name="persist", bufs=1))
    x16_pool = ctx.enter_context(tc.tile_pool(name="x16", bufs=3))
    qk_sb_pool = ctx.enter_context(tc.tile_pool(name="qksb", bufs=3))
    exp_pool = ctx.enter_context(tc.tile_pool(name="expt", bufs=3))
    ot_pool = ctx.enter_context(tc.tile_pool(name="ot", bufs=3))
    otT_pool = ctx.enter_context(tc.tile_pool(name="otT", bufs=3))
    rc_pool = ctx.enter_context(tc.tile_pool(name="rc", bufs=3))
    o_pool = ctx.enter_context(tc.tile_pool(name="osb", bufs=3))

    qk_ps_pool = ctx.enter_context(tc.tile_pool(name="qkps", bufs=1, space="PSUM"))
    dots_pool = ctx.enter_context(tc.tile_pool(name="dots", bufs=1, space="PSUM"))
    sm_pool = ctx.enter_context(tc.tile_pool(name="smps", bufs=2, space="PSUM"))

    # ---- persistent tiles ----
    xT = persist.tile([128, T * T], F16)        # x_b^T: [d, h*128+w]
    wq_sb = [[persist.tile([128, D], F16, name=f"wq{p}{eo}") for eo in range(2)]
             for p in range(2)]
    wk_sb = [[persist.tile([128, D], F16, name=f"wk{p}{eo}") for eo in range(2)]
             for p in range(2)]
    wv_sb = [persist.tile([128, D], F16, name=f"wv{p}") for p in range(2)]
    wo_sb = [persist.tile([128, D], F16, name=f"wo{p}") for p in range(2)]
    bo_sb = [persist.tile([1, D], F16, name=f"bo{p}") for p in range(2)]
    ones1 = persist.tile([1, D], F16)
    id_sb = persist.tile([D, D], F16)
    vext = [persist.tile([128, 17 * HEADS], F16, name=f"vext{k}") for k in range(2)]
    acc0 = persist.tile([128, T * D], F16)   # pass0 finals: [h, (w d)]
    accT = persist.tile([128, T * D], F16)   # transposed:   [w, (d h)]
    nc.sync.dma_start(out=id_sb[:, :], in_=ident[:, :])

    for p in range(2):
        for eo in range(2):
            nc.sync.dma_start(out=wq_sb[p][eo][:, :], in_=wq[p, eo, :, :])
            nc.sync.dma_start(out=wk_sb[p][eo][:, :], in_=wk[p, eo, :, :])
        nc.sync.dma_start(out=wv_sb[p][:, :], in_=wv[p, :, :])
        nc.sync.dma_start(out=wo_sb[p][:, :], in_=wo[p, :, :])
        nc.sync.dma_start(out=bo_sb[p][:, :], in_=bo[p, :, :])
    nc.vector.memset(ones1[:, :], 1.0)
    for k in range(2):
        nc.vector.memset(vext[k][:, :], 0.0)
        nc.vector.memset(
            vext[k][:, :].rearrange("p (h q) -> p h q", q=17)[:, :, 16:17], 1.0
        )

    # ---- Phase 0: build xT (transpose x into channel-major, fp16) ----
    # One persistent landing buffer with 16 disjoint-region loads: no WAW/WAR
    # deps on the DMAs (DMA descriptors only support ~2 sync waits).
    xflat = x[:, :, :].rearrange("h w d -> (h w) d")
    xld = persist.tile([128, 8, 8, 128], F32)
    for t in range(16):
        # rows [1024*t, 1024*(t+1)) as [128 partitions, 8 blocks, 128 d]
        src = bass.AP(
            tensor=xflat.tensor,
            offset=xflat.offset + t * 1024 * D,
            ap=[[D, 128], [128 * D, 8], [1, D]],
        )
        nc.sync.dma_start(out=xld[:, t % 8, :, :], in_=src)
        x16 = x16_pool.tile([128, 8, 128], F16)
        nc.vector.tensor_copy(out=x16[:, :, :], in_=xld[:, t % 8, :, :])
        tp = dots_pool.tile([128, 1024], F32, name="tp0", tag="dots")
        tp16 = tp[:, :].bitcast(F16)           # [128, 2048] f16 view
        for j in range(8):
            nc.tensor.transpose(tp16[:, j * 128:(j + 1) * 128], x16[:, j, :],
                                id_sb[:, :])
        nc.vector.tensor_copy(out=xT[:, t * 1024:(t + 1) * 1024],
                              in_=tp16[:, 0:1024])

    xT_hw = xT[:, :].rearrange("p (h w) -> p h w", w=T)

    acc0v = acc0[:, :].rearrange("p (w d) -> p w d", d=D)
    accTv = accT[:, :].rearrange("p (d h) -> p d h", h=T)

    # ---- attention passes ----
    for p in range(2):
        if p == 1:
            # inter-pass on-chip transpose: acc0 [h, (w d)] -> accT [w, (d h)]
            # via 128 per-channel PE transposes of the [h, w] planes.
            for d0 in range(0, D, 8):
                tpt = dots_pool.tile([128, 1024], F32, name="tpt", tag="dots")
                tpt16 = tpt[:, :].bitcast(F16)
                for j in range(8):
                    nc.tensor.transpose(tpt16[:, j * 128:(j + 1) * 128],
                                        acc0v[:, :, d0 + j], id_sb[:, :])
                nc.vector.tensor_copy(
                    out=accTv[:, d0:d0 + 8, :], in_=tpt16[:, 0:1024])
        for g in range((nseq + 1) // 2):
            seqs = [s for s in (2 * g, 2 * g + 1) if s < nseq]
            qk_ps = qk_ps_pool.tile([128, 1024], F32)
            for s2, s in enumerate(seqs):
                if p == 0:
                    xTs = xT_hw[:, :, s]          # attend along H: [d, h] strided
                else:
                    xTs = xT_hw[:, s, :]          # attend along W: [d, w] contig
                c0 = s2 * 512
                nc.tensor.matmul(qk_ps[:, c0 + 0:c0 + 128], wq_sb[p][0][:, :], xTs)
                nc.tensor.matmul(qk_ps[:, c0 + 128:c0 + 256], wq_sb[p][1][:, :], xTs)
                nc.tensor.matmul(qk_ps[:, c0 + 256:c0 + 384], wk_sb[p][0][:, :], xTs)
                nc.tensor.matmul(qk_ps[:, c0 + 384:c0 + 512], wk_sb[p][1][:, :], xTs)
            qk_sb = qk_sb_pool.tile([128, 1024], F16)
            nc.vector.tensor_copy(out=qk_sb[:, :512 * len(seqs)],
                                  in_=qk_ps[:, :512 * len(seqs)])

            # dots^T per head via row-tiled K=16 matmuls. Concurrent row-tiled
            # matmuls that write the SAME PSUM bank crash the hardware; MMs in
            # the same row group serialize in the array, so bank = row group.
            # Column layout: 512*(h//2) + (h%2)*128 + s2*256.
            dots = dots_pool.tile([128, 2048], F32, tag="dots")
            sms = []
            for s2, s in enumerate(seqs):
                if p == 0:
                    xTs = xT_hw[:, :, s]
                else:
                    xTs = xT_hw[:, s, :]
                c0 = s2 * 512
                sm = sm_pool.tile([128, 512], F32)
                sms.append(sm)
                # v natural: [t, d'] = xTs.T @ Wv
                nc.tensor.matmul(sm[:, 0:128], xTs, wv_sb[p][:, :])
                vx = vext[s % 2]
                nc.vector.tensor_copy(
                    out=vx[:, :].rearrange("p (h q) -> p h q", q=17)[:, :, 0:16],
                    in_=sm[:, 0:128].rearrange("p (h e) -> p h e", e=16),
                )
                for c in range(4):
                    for eo in range(2):
                        qcol = c0 + eo * 128
                        kcol = c0 + 256 + eo * 128
                        dcol = 512 * c + eo * 128 + s2 * 256
                        nc.tensor.matmul(
                            dots[:, dcol:dcol + 128],
                            qk_sb[32 * c:32 * c + 16, kcol:kcol + 128],
                            qk_sb[32 * c:32 * c + 16, qcol:qcol + 128],
                            tile_position=(32 * c, 0),
                        )
            expT = exp_pool.tile([128, 2048], F16)
            nc.scalar.activation(
                out=expT[:, :], in_=dots[:, :],
                func=mybir.ActivationFunctionType.Exp, scale=0.25,
            )
            for s2, s in enumerate(seqs):
                sm = sms[s2]
                vx = vext[s % 2]
                # PV with ones-column -> values + denominators
                for h in range(8):
                    ecol = 512 * (h // 2) + (h % 2) * 128 + s2 * 256
                    nc.tensor.matmul(
                        sm[:, 128 + 17 * h:128 + 17 * (h + 1)],
                        expT[:, ecol:ecol + 128],
                        vx[:, 17 * h:17 * (h + 1)],
                    )
                onat = sm[:, 128:264].rearrange("p (h q) -> p h q", q=17)
                rc = rc_pool.tile([128, 8, 1], F32)
                nc.vector.reciprocal(out=rc[:, :, :], in_=onat[:, :, 16:17])
                ot = ot_pool.tile([128, 128], F16)
                rc_ap = rc[:, :, 0]
                rc_bcast = bass.AP(
                    tensor=rc_ap.tensor, offset=rc_ap.offset,
                    ap=[rc_ap.ap[0], [1, 8], [0, 16]],
                )
                nc.vector.tensor_tensor(
                    out=ot[:, :].rearrange("p (h e) -> p h e", e=16),
                    in0=onat[:, :, 0:16],
                    in1=rc_bcast,
                    op=mybir.AluOpType.mult,
                )
                otT_ps = sm[:, 392:456].bitcast(F16)   # [128, 128] f16 in-bank
                nc.tensor.transpose(otT_ps, ot[:, :], id_sb[:, :])
                otT = otT_pool.tile([128, 128], F16)
                nc.vector.tensor_copy(out=otT[:, :], in_=otT_ps)
                # final projection + bias
                nc.tensor.matmul(sm[:, 264:392], otT[:, :], wo_sb[p][:, :],
                                 start=True, stop=False)
                nc.tensor.matmul(sm[:, 264:392], ones1[:, :], bo_sb[p][:, :],
                                 start=False, stop=True)
                if p == 0:
                    nc.vector.tensor_copy(out=acc0v[:, s, :], in_=sm[:, 264:392])
                else:
                    o = o_pool.tile([128, 128], F32)
                    nc.vector.tensor_add(out=o[:, :], in0=sm[:, 264:392],
                                         in1=accTv[:, :, s])
                    nc.sync.dma_start(out=out[s, :, :], in_=o[:, :])


def build_nc(nseq: int = T) -> bass.Bass:
    nc = bacc.Bacc(trn_type="TRN2")
    with tile.TileContext(nc) as tc:
        with ExitStack() as ctx:
            _axial_body(ctx, tc, nseq)
    nc.compile()
    return nc


def prep_weights(Wq0, Wkv0, Wo0, bo0, Wq1, Wkv1, Wo1, bo1):
    """Host-side weight preprocessing -> fp16 device layouts."""
    wq = np.zeros((2, 2, D, D), np.float16)
    wk = np.zeros((2, 2, D, D), np.float16)
    wv = np.zeros((2, D, D), np.float16)
    wo = np.zeros((2, D, D), np.float16)
    bo = np.zeros((2, 1, D), np.float16)
    for p, (Wq, Wkv, Wo, bov) in enumerate(
        [(Wq0, Wkv0, Wo0, bo0), (Wq1, Wkv1, Wo1, bo1)]
    ):
        Wqf = np.asarray(Wq, np.float32)
        Wkf = np.asarray(Wkv, np.float32)[:, :D]
        Wvf = np.asarray(Wkv, np.float32)[:, D:]
        for c in range(4):
            for eo in range(2):
                h = 2 * c + eo
                wq[p, eo][:, 32 * c:32 * c + 16] = Wqf[:, 16 * h:16 * h + 16]
                wk[p, eo][:, 32 * c:32 * c + 16] = Wkf[:, 16 * h:16 * h + 16]
        wv[p] = Wvf.astype(np.float16)
        wo[p] = np.asarray(Wo, np.float32).astype(np.float16)
        bo[p, 0] = np.asarray(bov, np.float32).astype(np.float16)
    return dict(wq=wq, wk=wk, wv=wv, wo=wo, bo=bo)


_NC_CACHE = {}


def _get_nc(nseq: int = T) -> bass.Bass:
    if nseq not in _NC_CACHE:
        _NC_CACHE[nseq] = build_nc(nseq)
    return _NC_CACHE[nseq]


def kernel(x, Wq0, Wkv0, Wo0, bo0, Wq1, Wkv1, Wo1, bo1, _trace=False):
    x = np.asarray(x, np.float32)
    B = x.shape[0]
    assert B == N_CORES and x.shape[1:] == (T, T, D)
    w = prep_weights(Wq0, Wkv0, Wo0, bo0, Wq1, Wkv1, Wo1, bo1)
    w["ident"] = np.eye(D, dtype=np.float16)
    nc = _get_nc(T)
    in_maps = [dict(x=np.ascontiguousarray(x[c]), **w) for c in range(N_CORES)]
    res = run_bass_kernel_spmd(nc, in_maps, core_ids=list(range(N_CORES)),
                               trace=_trace)
    out = np.stack([res.results[c]["out"] for c in range(N_CORES)])
    if _trace:
        kernel.last_results = res
    return out.astype(np.float32)

